# revision 19
# baseline (speedup 1.0000x reference)
"""Trainium2 Bass kernel for nn_AdaptiveAttentionLoss (weighted-CE segment mean).

reference semantics (C=2, G=4096, BETA=2):
    ce  = logsumexp(x) - x[label]
    p   = exp(-ce)
    s   = (1 - p^2) * ce          # per-sample weighted CE
    out = mean_over_present_groups( segment_mean(s, index) )

Strategy: data-parallel over the sample dim on 8 NeuronCores. Host repacks
inputs (x -> planar bf16, index/label -> int16) to halve HBM traffic and
keep every DVE operand packed (2x mode). Each core:
  - streams its shard, computes s elementwise (ACT exp/ln chain, DVE mults)
  - segment-reduce via two-level one-hots: index = 64*hi + lo, accumulated
    by one PE matmul per 128-sample column into 4 round-robin PSUM tiles:
    hist[{cnt,val}*64h, 64l] += [oh_hi | oh_hi*s]^T @ oh_lo.
    The one-hot build runs on the DVE as three batched TensorTensor ops
    per 32 columns (2x perf mode; the Pool engine rejects TensorTensor and
    its per-instruction launch overhead (~1.3us) rules out per-column ops).
  - AllReduce of the [128, 64] stats across cores, then the masked
    group-mean average on-chip; all cores emit the same scalar.
"""

from contextlib import ExitStack

import numpy as np

import concourse.bass as bass
import concourse.tile as tile
from concourse import bacc, bass_isa, mybir
from concourse.bass_utils import run_bass_kernel_spmd

F32 = mybir.dt.float32
BF16 = mybir.dt.bfloat16
I32 = mybir.dt.int32
I16 = mybir.dt.int16

N_FULL = 16777216
G = 4096
CORES = 8
P = 128
H = 64  # hi bins (index >> 6)
L = 64  # lo bins (index & 63)
NBANK = 4  # PSUM accumulators round-robin

AX = mybir.AxisListType
OP = mybir.AluOpType
ACTF = mybir.ActivationFunctionType

RB = 64  # sample-columns per one-hot batch


def build_nc(n_core: int, chunk_f: int):
    """Build the SPMD Bass graph for one core holding n_core samples."""
    assert n_core % (P * chunk_f) == 0
    ftot = n_core // P
    nchunk = ftot // chunk_f

    nc = bacc.Bacc("TRN2", target_bir_lowering=False, debug=False)

    # planar x: x0 plane then x1 plane, each [n_core] bf16
    x_d = nc.declare_dram_parameter("x", [2, n_core], BF16, isOutput=False)
    idx_d = nc.declare_dram_parameter("index", [n_core], I16, isOutput=False)
    lab_d = nc.declare_dram_parameter("label", [n_core], I16, isOutput=False)
    out_d = nc.declare_dram_parameter("out", [1, 1], F32, isOutput=True)

    cc_in = nc.dram_tensor("cc_in", [P, L], F32)
    cc_out = nc.dram_tensor("cc_out", [P, L], F32, addr_space="Shared")

    x_v = x_d.ap().rearrange("c (p f) -> c p f", p=P)  # [2, 128, ftot]
    idx_v = idx_d.ap().rearrange("(p f) -> p f", p=P)
    lab_v = lab_d.ap().rearrange("(p f) -> p f", p=P)

    with tile.TileContext(nc) as tc, ExitStack() as ctx:
        const_pool = ctx.enter_context(tc.tile_pool(name="const", bufs=1))
        in_pool = ctx.enter_context(tc.tile_pool(name="inp", bufs=3))
        scr_pool = ctx.enter_context(tc.tile_pool(name="scr", bufs=2))
        oh_pool = ctx.enter_context(tc.tile_pool(name="oh", bufs=3))
        fin_pool = ctx.enter_context(tc.tile_pool(name="fin", bufs=1))
        psum_pool = ctx.enter_context(
            tc.tile_pool(name="psum", bufs=1, space="PSUM")
        )

        # bin-major iota constant: int16 value h replicated RB times
        iotw = const_pool.tile([P, H * RB], I16)
        nc.gpsimd.iota(iotw[:], pattern=[[1, H], [0, RB]], base=0,
                       channel_multiplier=0)
        iota_rep = iotw[:].rearrange("p (h r) -> p h r", r=RB)

        hists = []
        for i in range(NBANK):
            hist_i = psum_pool.tile([P, L], F32, tag=f"h{i}", name=f"hist_{i}")
            hists.append(hist_i)

        n_tiles_total = ftot  # one matmul per free column
        tile_no = 0

        for c in range(nchunk):
            sl = slice(c * chunk_f, (c + 1) * chunk_f)
            xt0 = in_pool.tile([P, chunk_f], BF16, tag="x0")
            xt1 = in_pool.tile([P, chunk_f], BF16, tag="x1")
            it = in_pool.tile([P, chunk_f], I16, tag="idx")
            lt = in_pool.tile([P, chunk_f], I16, tag="lab")
            nc.sync.dma_start(out=xt0[:], in_=x_v[0, :, sl])
            nc.sync.dma_start(out=xt1[:], in_=x_v[1, :, sl])
            nc.sync.dma_start(out=it[:], in_=idx_v[:, sl])
            nc.sync.dma_start(out=lt[:], in_=lab_v[:, sl])

            d = scr_pool.tile([P, chunk_f], BF16, tag="d")
            sign = scr_pool.tile([P, chunk_f], BF16, tag="sign")
            t = scr_pool.tile([P, chunk_f], BF16, tag="t")
            e = scr_pool.tile([P, chunk_f], F32, tag="e")
            ce = scr_pool.tile([P, chunk_f], BF16, tag="ce")
            p2 = scr_pool.tile([P, chunk_f], F32, tag="p2")
            wm = scr_pool.tile([P, chunk_f], BF16, tag="wm")
            sv = scr_pool.tile([P, chunk_f], BF16, tag="sv")
            hi16 = scr_pool.tile([P, chunk_f], I16, tag="hi16")
            lo16 = scr_pool.tile([P, chunk_f], I16, tag="lo16")

            # sign = 1 - 2*label   (int16 read converted by the fp32 ALU)
            nc.scalar.activation(sign[:], lt[:], ACTF.Identity, bias=1.0,
                                 scale=-2.0)
            nc.vector.tensor_tensor(out=d[:], in0=xt0[:], in1=xt1[:],
                                    op=OP.subtract)
            nc.vector.tensor_tensor(out=t[:], in0=d[:], in1=sign[:],
                                    op=OP.mult)
            # e = exp(-t); ce = ln(1+e); p2 = exp(-2 ce); s = (1-p2)*ce
            nc.scalar.activation(e[:], t[:], ACTF.Exp, scale=-1.0)
            nc.scalar.activation(ce[:], e[:], ACTF.Ln, bias=1.0)
            nc.scalar.activation(p2[:], ce[:], ACTF.Exp, scale=-2.0)
            nc.scalar.activation(wm[:], p2[:], ACTF.Identity, bias=1.0,
                                 scale=-1.0)
            nc.vector.tensor_tensor(out=sv[:], in0=wm[:], in1=ce[:],
                                    op=OP.mult)
            # hi = index >> 6 (int16 and f32 forms), lo = index & 63
            nc.vector.tensor_scalar(
                out=hi16[:], in0=it[:], scalar1=6, scalar2=None,
                op0=OP.logical_shift_right,
            )
            nc.vector.tensor_scalar(
                out=lo16[:], in0=it[:], scalar1=63, scalar2=None,
                op0=OP.bitwise_and,
            )

            # Histogram one-hot batches (bin-major, innermost step-1).
            rb = min(RB, chunk_f)
            for b in range(chunk_f // rb):
                bsl = slice(b * rb, (b + 1) * rb)
                ohb = oh_pool.tile([P, 2, H, rb], BF16, tag="ohb")
                olb = oh_pool.tile([P, H, rb], BF16, tag="olb")
                iota_b = iota_rep[:, :, 0:rb]
                hi_rep = hi16[:, bsl].unsqueeze(1).broadcast_to((P, H, rb))
                lo_rep = lo16[:, bsl].unsqueeze(1).broadcast_to((P, H, rb))
                sv_rep = sv[:, bsl].unsqueeze(1).broadcast_to((P, H, rb))
                nc.vector.tensor_tensor(
                    out=olb[:], in0=iota_b, in1=lo_rep, op=OP.is_equal
                )
                nc.vector.tensor_tensor(
                    out=ohb[:, 0, :, :], in0=iota_b, in1=hi_rep,
                    op=OP.is_equal,
                )
                nc.vector.tensor_tensor(
                    out=ohb[:, 1, :, :], in0=ohb[:, 0, :, :], in1=sv_rep,
                    op=OP.mult,
                )
                for j in range(rb):
                    acc = hists[tile_no % NBANK]
                    nc.tensor.matmul(
                        out=acc[:], lhsT=ohb[:, :, :, j], rhs=olb[:, :, j],
                        start=(tile_no < NBANK),
                        stop=(tile_no >= n_tiles_total - NBANK),
                    )
                    tile_no += 1

        # ---- finalize: AllReduce the [128, 64] stats, then masked mean ----
        stats = fin_pool.tile([P, L], F32, tag="stats")
        nc.vector.tensor_copy(out=stats[:], in_=hists[0][:])
        for _h in hists[1:]:
            nc.vector.tensor_tensor(out=stats[:], in0=stats[:], in1=_h[:],
                                    op=OP.add)
        nc.sync.dma_start(out=cc_in.ap(), in_=stats[:])
        nc.gpsimd.collective_compute(
            "AllReduce",
            OP.add,
            ins=[cc_in.ap().opt()],
            outs=[cc_out.ap().opt()],
            replica_groups=[list(range(CORES))],
        )
        cnt_t = fin_pool.tile([H, L], F32, tag="cnt_t")
        val_t = fin_pool.tile([H, L], F32, tag="val_t")
        cc_v = cc_out.ap()
        nc.sync.dma_start(out=cnt_t[:], in_=cc_v[0:H, :])
        nc.sync.dma_start(out=val_t[:], in_=cc_v[H : 2 * H, :])
        cnt = cnt_t[:]
        val = val_t[:]
        cntc = fin_pool.tile([H, L], F32, tag="cntc")
        gm = fin_pool.tile([H, L], F32, tag="gm")
        pres = fin_pool.tile([H, L], F32, tag="pres")
        nc.vector.tensor_scalar(
            out=cntc[:], in0=cnt, scalar1=1.0, scalar2=None, op0=OP.max
        )
        nc.vector.reciprocal(out=cntc[:], in_=cntc[:])
        nc.vector.tensor_tensor(out=gm[:], in0=val, in1=cntc[:], op=OP.mult)
        nc.vector.tensor_scalar(
            out=pres[:], in0=cnt, scalar1=0.0, scalar2=None, op0=OP.is_gt
        )
        nc.vector.tensor_tensor(out=gm[:], in0=gm[:], in1=pres[:], op=OP.mult)

        # free-axis reduce on DVE, then partition all-reduce on GPSIMD
        red2 = fin_pool.tile([H, 2], F32, tag="red2")
        nc.vector.tensor_reduce(out=red2[:, 0:1], in_=gm[:], axis=AX.XYZW,
                                op=OP.add)
        nc.vector.tensor_reduce(out=red2[:, 1:2], in_=pres[:], axis=AX.XYZW,
                                op=OP.add)
        red2r = fin_pool.tile([H, 2], F32, tag="red2r")
        nc.gpsimd.partition_all_reduce(
            red2r[:], red2[:], channels=H, reduce_op=bass_isa.ReduceOp.add
        )
        ans = fin_pool.tile([1, 1], F32, tag="ans")
        recip = fin_pool.tile([1, 1], F32, tag="recip")
        nc.vector.reciprocal(out=recip[:], in_=red2r[0:1, 1:2])
        nc.vector.tensor_tensor(out=ans[:], in0=red2r[0:1, 0:1], in1=recip[:],
                                op=OP.mult)
        nc.sync.dma_start(out=out_d.ap(), in_=ans[:])

    nc.finalize()
    return nc


def make_in_maps(x, index, label, n_cores=CORES):
    n = x.shape[0]
    nc_sz = n // n_cores
    # host-side dtype repack: x -> planar bf16 (round-to-nearest-even),
    # index/label -> int16. Halves HBM traffic; values are exact for
    # index (< 4096) and label (0/1).
    import ml_dtypes

    xb = np.ascontiguousarray(
        np.asarray(x, dtype=np.float32).T
    ).astype(ml_dtypes.bfloat16)  # [2, n]
    iv = np.asarray(index).astype(np.int16)
    lv = np.asarray(label).astype(np.int16)
    maps = []
    for k in range(n_cores):
        sl = slice(k * nc_sz, (k + 1) * nc_sz)
        maps.append(
            {
                "x": np.ascontiguousarray(xb[:, sl]),
                "index": np.ascontiguousarray(iv[sl]),
                "label": np.ascontiguousarray(lv[sl]),
            }
        )
    return maps


_NC_CACHE = {}


def _get_nc(n_core, chunk_f):
    key = (n_core, chunk_f)
    if key not in _NC_CACHE:
        _NC_CACHE[key] = build_nc(n_core, chunk_f)
    return _NC_CACHE[key]


def kernel(x, index, label):
    n = x.shape[0]
    n_core = n // CORES
    nc = _get_nc(n_core, min(1024, n_core // P))
    in_maps = make_in_maps(x, index, label)
    res = run_bass_kernel_spmd(nc, in_maps, core_ids=list(range(CORES)))
    return np.float32(res.results[0]["out"][0, 0])


if __name__ == "__main__":
    rng = np.random.default_rng(0)
    n = 128 * 32 * CORES
    x = rng.standard_normal((n, 2), dtype=np.float32)
    index = rng.integers(0, G, n, dtype=np.int64)
    label = rng.integers(0, 2, n, dtype=np.int64)
    got = kernel(x, index, label)
    # numpy reference
    m = np.maximum(x[:, 0], x[:, 1])
    logz = m + np.log(np.exp(x[:, 0] - m) + np.exp(x[:, 1] - m))
    xt = x[np.arange(n), label]
    ce = logz - xt
    p = np.exp(xt - logz)
    s = (1.0 - p**2) * ce
    seg = np.zeros(G)
    cntr = np.zeros(G)
    np.add.at(seg, index, s)
    np.add.at(cntr, index, 1.0)
    pres = cntr > 0
    gmean = np.where(pres, seg / np.maximum(cntr, 1), 0.0)
    want = gmean.sum() / pres.sum()
    print("got", got, "want", want, "rel", abs(got - want) / abs(want))
